# revision 13
# baseline (speedup 1.0000x reference)
"""AdaPT Linear (int8 systolic fake-quant matmul) on 8 TRN2 NeuronCores.

Reference semantics (single device):
    amax_x = max|x|, amax_w = max|w|         (global scalars)
    sx = 127/amax_x, sw = 127/amax_w
    qx = round(x*sx)  (int8), qw = round(w*sw)  (int8)
    out = (qx @ qw.T)_int32 / (sx*sw) + bias

Distribution:
  - x: data-parallel over rows (8 x 1024); each core computes its out rows.
  - weight: tensor-parallel over the K (contraction) axis for quantization:
    core c owns w[:, c*512:(c+1)*512], quantizes + transposes it on-chip
    (TensorE 128x128 transposes -> PSUM -> fp32 magic-number round), then 8
    pipelined AllGathers (one per 512-col output block) assemble the full
    transposed quantized weight [4096k, 512n] per block -- the axis-0 concat
    of per-core [512k, 512n] pieces IS the k-major layout the matmul needs.
  - amax: per-core partials + two tiny AllGathers (w first, then x).
  - x quantization: fp32 magic-number round (bit-exact round-half-to-even),
    int8 values stored in bf16, transposed k-onto-partitions via the XBAR
    DMA transpose through a DRAM scratch.
  - bf16 matmul accumulates in fp32 PSUM: int8 products (<2^14) and sums
    (<2^24) are exact, so this reproduces the int8 systolic MAC.
  - epilogue: out = psum * (1/(sx*sw)) + bias (one DVE op), DMA out.

One NEFF; Tile generates all semaphores.
"""

import numpy as np

P = 128
MAGIC = 12582912.0  # 1.5 * 2**23: fp32 RNE round-to-int trick
MAXV = 127.0
NCORES = 8

# full-problem shapes (hardcoded per the task)
FULL_B, FULL_S, FULL_K = 4, 2048, 4096
FULL_N = 4096


def build_graph(M=1024, N=4096, K=4096, ncores=NCORES):
    """Build the SPMD Bass graph for one core (identical on all cores)."""
    import concourse.bass as bass
    import concourse.mybir as mybir
    import concourse.tile as tile
    from concourse import bacc, bass_isa
    from concourse.masks import make_identity

    assert M % P == 0 and K % P == 0 and N % 512 == 0
    KSL = K // ncores       # k-columns of w owned per core (512)
    KLT = KSL // P          # local k tiles (4)
    KT = K // P             # global k tiles
    MB = M // P             # m blocks
    NB = N // 512           # n blocks of 512
    KC = min(2048, K)       # free-dim chunk for streaming x
    KH = K // KC
    G = min(8, KC // P)     # k-tiles per x group (unused for w now)

    f32 = mybir.dt.float32
    bf16 = mybir.dt.bfloat16

    nc = bacc.Bacc(None, num_devices=ncores)

    x_ext = nc.declare_dram_parameter("x", [M, K], f32, isOutput=False)
    # w k-slice per core: w[:, c*KSL:(c+1)*KSL], contiguous [N, KSL]
    wsl_ext = nc.declare_dram_parameter("wslice", [N, KSL], f32, isOutput=False)
    b_ext = nc.declare_dram_parameter("bias", [N], f32, isOutput=False)
    out_ext = nc.declare_dram_parameter("out", [M, N], f32, isOutput=True)

    qx_dram = nc.dram_tensor("qx_scratch", [M, K], bf16)
    ccw_in = nc.dram_tensor("ccw_in", [1, 1], f32)
    ccw_out = nc.dram_tensor("ccw_out", [ncores, 1], f32)
    ccx_in = nc.dram_tensor("ccx_in", [1, 1], f32)
    ccx_out = nc.dram_tensor("ccx_out", [ncores, 1], f32)
    qw_ins = [nc.dram_tensor(f"qw_in{nb}", [KSL, 512], bf16) for nb in range(NB)]
    qw_outs = [nc.dram_tensor(f"qw_out{nb}", [K, 512], bf16, addr_space="Shared")
               for nb in range(NB)]

    with tile.TileContext(nc) as tc:
        with (
            tc.tile_pool(name="xchunks", bufs=2) as xpool,
            tc.tile_pool(name="wchunks", bufs=2) as wpool,
            tc.tile_pool(name="qxc", bufs=2) as qxpool,
            tc.tile_pool(name="persist", bufs=1) as persist,
            tc.tile_pool(name="qwt", bufs=2) as qwtpool,
            tc.tile_pool(name="qwl", bufs=2) as qwlpool,
            tc.tile_pool(name="tw", bufs=2) as twpool,
            tc.tile_pool(name="ob", bufs=3) as obpool,
            tc.tile_pool(name="stats", bufs=1) as stats,
            tc.tile_pool(name="psum_tr", bufs=3, space="PSUM") as pstr,
            tc.tile_pool(name="psum_mm", bufs=2, space="PSUM") as psmm,
        ):
            rg = [list(range(ncores))]

            # ---------- Phase A1: w-slice amax (DVE head) + exchange ----------
            # wslice viewed as [N/P, P, KSL]; big strided loads of 4 blocks
            WBL = 4 * KSL               # free elems per partition per load
            wmaxes = stats.tile([P, N // (4 * P)], f32)
            wsl_v = wsl_ext[:].rearrange("(a p) k -> a p k", p=P)
            for i in range(N // (4 * P)):
                wc = wpool.tile([P, 4, KSL], f32)
                nc.sync.dma_start(out=wc, in_=wsl_v[4 * i:4 * (i + 1)].rearrange("a p k -> p a k"))
                nc.vector.tensor_reduce(
                    out=wmaxes[:, i:i + 1], in_=wc,
                    axis=mybir.AxisListType.XY, op=mybir.AluOpType.max,
                    apply_absolute_value=True)
            wmax_v = stats.tile([P, 1], f32)
            nc.vector.tensor_reduce(out=wmax_v, in_=wmaxes, axis=mybir.AxisListType.X,
                                    op=mybir.AluOpType.max)
            wmax_p = stats.tile([P, 1], f32)
            nc.gpsimd.partition_all_reduce(wmax_p, wmax_v, channels=P,
                                           reduce_op=bass_isa.ReduceOp.max)
            nc.sync.dma_start(out=ccw_in[:], in_=wmax_p[0:1, :])
            nc.gpsimd.collective_compute(
                "AllGather", mybir.AluOpType.bypass, replica_groups=rg,
                ins=[ccw_in[:].opt()], outs=[ccw_out[:].opt()])
            gat_w = stats.tile([ncores, 1], f32)
            nc.sync.dma_start(out=gat_w, in_=ccw_out[:])
            gmax_w = stats.tile([ncores, 1], f32)
            nc.gpsimd.partition_all_reduce(gmax_w, gat_w, channels=ncores,
                                           reduce_op=bass_isa.ReduceOp.max)
            aw = gmax_w[0:1, 0:1]

            # ---------- Phase A2: x amax + exchange ----------
            xmaxes = stats.tile([P, MB * KH], f32)
            for i in range(MB):
                for h in range(KH):
                    xc = xpool.tile([P, KC], f32)
                    nc.sync.dma_start(out=xc, in_=x_ext[i * P:(i + 1) * P, h * KC:(h + 1) * KC])
                    nc.vector.tensor_reduce(
                        out=xmaxes[:, i * KH + h:i * KH + h + 1], in_=xc,
                        axis=mybir.AxisListType.X, op=mybir.AluOpType.max,
                        apply_absolute_value=True)
            xmax_v = stats.tile([P, 1], f32)
            nc.vector.tensor_reduce(out=xmax_v, in_=xmaxes, axis=mybir.AxisListType.X,
                                    op=mybir.AluOpType.max)
            xmax_p = stats.tile([P, 1], f32)
            nc.gpsimd.partition_all_reduce(xmax_p, xmax_v, channels=P,
                                           reduce_op=bass_isa.ReduceOp.max)
            nc.sync.dma_start(out=ccx_in[:], in_=xmax_p[0:1, :])
            nc.gpsimd.collective_compute(
                "AllGather", mybir.AluOpType.bypass, replica_groups=rg,
                ins=[ccx_in[:].opt()], outs=[ccx_out[:].opt()])
            gat_x = stats.tile([ncores, 1], f32)
            nc.sync.dma_start(out=gat_x, in_=ccx_out[:])
            gmax_x = stats.tile([ncores, 1], f32)
            nc.gpsimd.partition_all_reduce(gmax_x, gat_x, channels=ncores,
                                           reduce_op=bass_isa.ReduceOp.max)
            ax = gmax_x[0:1, 0:1]

            # ---------- scales ----------
            scw = stats.tile([1, 4], f32)
            scx = stats.tile([1, 4], f32)
            sx_t = stats.tile([1, 1], f32)
            sw_t = stats.tile([1, 1], f32)
            ds_t = stats.tile([1, 1], f32)
            dsc = stats.tile([1, 4], f32)

            def recip(dst, src, t0, t1):
                nc.vector.reciprocal(dst, src)
                nc.vector.tensor_tensor(out=t0, in0=src, in1=dst,
                                        op=mybir.AluOpType.mult)
                nc.vector.tensor_scalar(out=t1, in0=t0, scalar1=-1.0, scalar2=2.0,
                                        op0=mybir.AluOpType.mult,
                                        op1=mybir.AluOpType.add)
                nc.vector.tensor_tensor(out=dst, in0=dst, in1=t1,
                                        op=mybir.AluOpType.mult)

            recip(scw[0:1, 0:1], aw, scw[0:1, 1:2], scw[0:1, 2:3])
            nc.vector.tensor_scalar(out=sw_t, in0=scw[0:1, 0:1], scalar1=MAXV,
                                    scalar2=None, op0=mybir.AluOpType.mult)
            swb = stats.tile([P, 1], f32)
            nc.gpsimd.partition_broadcast(swb, sw_t)

            recip(scx[0:1, 0:1], ax, scx[0:1, 1:2], scx[0:1, 2:3])
            nc.vector.tensor_scalar(out=sx_t, in0=scx[0:1, 0:1], scalar1=MAXV,
                                    scalar2=None, op0=mybir.AluOpType.mult)
            sxb = stats.tile([P, 1], f32)
            nc.gpsimd.partition_broadcast(sxb, sx_t)

            nc.vector.tensor_tensor(out=dsc[0:1, 0:1], in0=sx_t, in1=sw_t,
                                    op=mybir.AluOpType.mult)
            recip(ds_t, dsc[0:1, 0:1], dsc[0:1, 1:2], dsc[0:1, 2:3])
            dsb = stats.tile([P, 1], f32)
            nc.gpsimd.partition_broadcast(dsb, ds_t)

            # bias replicated into all partitions (fp32)
            bias_t = persist.tile([P, N], f32)
            bias_bcast = bass.AP(tensor=b_ext, offset=0, ap=[[0, P], [1, N]])
            nc.sync.dma_start(out=bias_t, in_=bias_bcast)

            ident = persist.tile([P, P], f32)
            make_identity(nc, ident[:])

            # ---------- Phase B: per-block w quantize+transpose + AllGather ----------
            for nb in range(NB):
                qwl = qwlpool.tile([P, KLT, 512], bf16)   # local [k 512, n 512]
                for s in range(4):                          # 128 n-rows each
                    wc = wpool.tile([P, 4, KSL], f32)       # reuse pool (same shape)
                    wcs = wc[:, 0, :]                       # [128, KSL] view
                    nc.sync.dma_start(out=wcs,
                                      in_=wsl_ext[nb * 512 + s * P: nb * 512 + (s + 1) * P, :])
                    ps = pstr.tile([P, KLT, P], f32, space="PSUM")
                    for kl in range(KLT):
                        nc.tensor.transpose(ps[:, kl, :], wcs[:, kl * P:(kl + 1) * P],
                                            ident[:])
                    twt = twpool.tile([P, KLT, P], f32)
                    nc.scalar.activation(out=twt, in_=ps,
                                         func=mybir.ActivationFunctionType.Copy,
                                         bias=MAGIC, scale=swb)
                    nc.vector.tensor_scalar(
                        out=qwl[:, :, s * P:(s + 1) * P],
                        in0=twt, scalar1=-MAGIC, scalar2=None,
                        op0=mybir.AluOpType.add)
                qwin_v = qw_ins[nb][:].rearrange("(kl p) n -> p kl n", p=P)
                nc.sync.dma_start(out=qwin_v, in_=qwl[:])
                nc.gpsimd.collective_compute(
                    "AllGather", mybir.AluOpType.bypass, replica_groups=rg,
                    ins=[qw_ins[nb][:].opt()], outs=[qw_outs[nb][:].opt()])

            # ---------- Phase C: quantize + transpose x ----------
            for i in range(MB):
                for h in range(KH):
                    xc = xpool.tile([P, KC], f32)
                    nc.sync.dma_start(out=xc, in_=x_ext[i * P:(i + 1) * P, h * KC:(h + 1) * KC])
                    nc.vector.tensor_scalar(out=xc, in0=xc, scalar1=sxb,
                                            scalar2=MAGIC, op0=mybir.AluOpType.mult,
                                            op1=mybir.AluOpType.add)
                    qc = qxpool.tile([P, KC], bf16)
                    nc.scalar.activation(out=qc, in_=xc,
                                         func=mybir.ActivationFunctionType.Copy,
                                         bias=-MAGIC, scale=1.0)
                    nc.sync.dma_start(out=qx_dram[i * P:(i + 1) * P, h * KC:(h + 1) * KC],
                                      in_=qc)
            # XBAR transpose: qxT[p, kt, m] = qx[m, kt*128+p]
            qxT = persist.tile([P, KT, M], bf16)
            MTR = min(M, 512)
            for kt in range(KT):
                for mh in range(M // MTR):
                    nc.sync.dma_start(
                        out=qxT[:, kt, mh * MTR:(mh + 1) * MTR],
                        in_=qx_dram[mh * MTR:(mh + 1) * MTR, kt * P:(kt + 1) * P],
                        transpose=True)

            # ---------- Phase D: matmul per gathered block ----------
            for nb in range(NB):
                qwT = qwtpool.tile([P, KT, 512], bf16)
                qwo_v = qw_outs[nb][:].rearrange("(kt p) n -> p kt n", p=P)
                nc.sync.dma_start(out=qwT[:], in_=qwo_v)
                for mb in range(MB):
                    acc = psmm.tile([P, 512], f32, space="PSUM")
                    for kt in range(KT):
                        nc.tensor.matmul(
                            acc, qxT[:, kt, mb * P:(mb + 1) * P], qwT[:, kt, :],
                            start=(kt == 0), stop=(kt == KT - 1))
                    ob = obpool.tile([P, 512], f32)
                    nc.vector.scalar_tensor_tensor(
                        out=ob, in0=acc, scalar=dsb,
                        in1=bias_t[:, nb * 512:(nb + 1) * 512],
                        op0=mybir.AluOpType.mult, op1=mybir.AluOpType.add)
                    nc.sync.dma_start(
                        out=out_ext[mb * P:(mb + 1) * P, nb * 512:(nb + 1) * 512],
                        in_=ob)
    nc.compile()
    return nc


def shard_inputs(x, weight, bias, M=1024, K=4096, ncores=NCORES):
    xf = np.ascontiguousarray(np.asarray(x, dtype=np.float32).reshape(-1, x.shape[-1]))
    w = np.asarray(weight, dtype=np.float32)
    b = np.ascontiguousarray(np.asarray(bias, dtype=np.float32))
    ksl = K // ncores
    in_maps = []
    for c in range(ncores):
        in_maps.append({
            "x": np.ascontiguousarray(xf[c * M:(c + 1) * M]),
            "wslice": np.ascontiguousarray(w[:, c * ksl:(c + 1) * ksl]),
            "bias": b,
        })
    return in_maps


def _run(x, weight, bias, trace=False):
    from concourse.bass_utils import run_bass_kernel_spmd

    nc = build_graph()
    in_maps = shard_inputs(x, weight, bias)
    res = run_bass_kernel_spmd(nc, in_maps, core_ids=list(range(NCORES)),
                               trace=trace)
    outs = [res.results[c]["out"] for c in range(NCORES)]
    full = np.concatenate(outs, axis=0).reshape(FULL_B, FULL_S, FULL_N)
    return full.astype(np.float32), res


def kernel(x, weight, bias):
    out, _ = _run(x, weight, bias, trace=False)
    return out


# revision 16
# speedup vs baseline: 1.0727x; 1.0727x over previous
"""AdaPT Linear (int8 systolic fake-quant matmul) on 8 TRN2 NeuronCores.

Reference semantics (single device):
    amax_x = max|x|, amax_w = max|w|         (global scalars)
    sx = 127/amax_x, sw = 127/amax_w
    qx = round(x*sx)  (int8), qw = round(w*sw)  (int8)
    out = (qx @ qw.T)_int32 / (sx*sw) + bias

Distribution: data-parallel over x rows (8 x 1024).  Each core:
  - computes partial amax over its x shard + its 512-row weight slice
  - two tiny AllGathers exchange the partial-amax scalars (amax_w first so
    the weight pipeline unblocks early)
  - quantizes x with an fp32 magic-number round (bit-exact round-half-even),
    stores int8-valued bf16, transposes via the XBAR DMA transpose
  - streams the full fp32 weight from HBM, transposes 128x128 tiles on the
    TensorEngine (f32r: 1.5 cyc/row), quantizes on the way out of PSUM
    (ACT pass1 + DVE pass2)
  - bf16 matmul accumulates in fp32 PSUM: int8 products (<2^14) and sums
    (<2^24) are exact, so this reproduces the int8 systolic MAC
  - epilogue: out = psum * (1/(sx*sw)) + bias (one DVE op), DMA to DRAM

The whole thing is one NEFF; Tile generates all semaphores.
"""

import numpy as np

P = 128
MAGIC = 12582912.0  # 1.5 * 2**23: fp32 RNE round-to-int trick
MAXV = 127.0
NCORES = 8

# full-problem shapes (hardcoded per the task)
FULL_B, FULL_S, FULL_K = 4, 2048, 4096
FULL_N = 4096


def build_graph(M=1024, N=4096, K=4096, ncores=NCORES):
    """Build the SPMD Bass graph for one core (identical on all cores)."""
    import concourse.bass as bass
    import concourse.mybir as mybir
    import concourse.tile as tile
    from concourse import bacc, bass_isa
    from concourse.masks import make_identity

    assert M % P == 0 and K % P == 0 and N % 512 == 0
    NSL = N // ncores      # weight slice rows per core (for amax)
    KT = K // P            # k tiles
    MB = M // P            # m blocks
    NB = N // 512          # n blocks of 512
    KC = min(2048, K)      # free-dim chunk for streaming f32
    KH = K // KC           # chunks per row-block of x / w
    G = min(8, KC // P)    # k-tiles per transpose/quant group

    f32 = mybir.dt.float32
    f32r = mybir.dt.float32r
    bf16 = mybir.dt.bfloat16

    nc = bacc.Bacc(None, num_devices=ncores)

    x_ext = nc.declare_dram_parameter("x", [M, K], f32, isOutput=False)
    w_ext = nc.declare_dram_parameter("w", [N, K], f32, isOutput=False)
    wsl_ext = nc.declare_dram_parameter("wslice", [NSL, K], f32, isOutput=False)
    b_ext = nc.declare_dram_parameter("bias", [N], f32, isOutput=False)
    out_ext = nc.declare_dram_parameter("out", [M, N], f32, isOutput=True)

    ccw_in = nc.dram_tensor("ccw_in", [1, 1], f32)
    ccw_out = nc.dram_tensor("ccw_out", [ncores, 1], f32)
    ccx_in = nc.dram_tensor("ccx_in", [1, 1], f32)
    ccx_out = nc.dram_tensor("ccx_out", [ncores, 1], f32)

    with tile.TileContext(nc) as tc:
        with (
            tc.tile_pool(name="xchunks", bufs=2) as xpool,
            tc.tile_pool(name="wchunks", bufs=2) as wpool,
            tc.tile_pool(name="qxc", bufs=2) as qxpool,
            tc.tile_pool(name="persist", bufs=1) as persist,
            tc.tile_pool(name="qwt", bufs=2) as qwtpool,
            tc.tile_pool(name="tw", bufs=2) as twpool,
            tc.tile_pool(name="ob", bufs=3) as obpool,
            tc.tile_pool(name="stats", bufs=1) as stats,
            tc.tile_pool(name="psum_tr", bufs=2, space="PSUM") as pstr,
            tc.tile_pool(name="psum_mm", bufs=2, space="PSUM") as psmm,
            tc.tile_pool(name="psum_x", bufs=2, space="PSUM") as psx,
        ):
            rg = [list(range(ncores))]

            # ---------- Phase A2: x amax + exchange ----------
            xmaxes = stats.tile([P, MB * KH], f32)
            for i in range(MB):
                for h in range(KH):
                    xc = xpool.tile([P, KC], f32)
                    nc.sync.dma_start(out=xc, in_=x_ext[i * P:(i + 1) * P, h * KC:(h + 1) * KC])
                    nc.vector.tensor_reduce(
                        out=xmaxes[:, i * KH + h:i * KH + h + 1], in_=xc,
                        axis=mybir.AxisListType.X, op=mybir.AluOpType.max,
                        apply_absolute_value=True)
            xmax_v = stats.tile([P, 1], f32)
            nc.vector.tensor_reduce(out=xmax_v, in_=xmaxes, axis=mybir.AxisListType.X,
                                    op=mybir.AluOpType.max)
            xmax_p = stats.tile([P, 1], f32)
            nc.gpsimd.partition_all_reduce(xmax_p, xmax_v, channels=P,
                                           reduce_op=bass_isa.ReduceOp.max)
            nc.sync.dma_start(out=ccx_in[:], in_=xmax_p[0:1, :])
            nc.gpsimd.collective_compute(
                "AllGather", mybir.AluOpType.bypass, replica_groups=rg,
                ins=[ccx_in[:].opt()], outs=[ccx_out[:].opt()])
            gat_x = stats.tile([ncores, 1], f32)
            nc.sync.dma_start(out=gat_x, in_=ccx_out[:])
            gmax_x = stats.tile([ncores, 1], f32)
            nc.gpsimd.partition_all_reduce(gmax_x, gat_x, channels=ncores,
                                           reduce_op=bass_isa.ReduceOp.max)
            ax = gmax_x[0:1, 0:1]

            # ---------- Phase A1: weight-slice amax + exchange ----------
            wmaxes = stats.tile([P, (NSL // P) * KH], f32)
            for i in range(NSL // P):
                for h in range(KH):
                    wc = wpool.tile([P, KC], f32)
                    nc.sync.dma_start(out=wc, in_=wsl_ext[i * P:(i + 1) * P, h * KC:(h + 1) * KC])
                    nc.vector.tensor_reduce(
                        out=wmaxes[:, i * KH + h:i * KH + h + 1], in_=wc,
                        axis=mybir.AxisListType.X, op=mybir.AluOpType.max,
                        apply_absolute_value=True)
            wmax_v = stats.tile([P, 1], f32)
            nc.vector.tensor_reduce(out=wmax_v, in_=wmaxes, axis=mybir.AxisListType.X,
                                    op=mybir.AluOpType.max)
            wmax_p = stats.tile([P, 1], f32)
            nc.gpsimd.partition_all_reduce(wmax_p, wmax_v, channels=P,
                                           reduce_op=bass_isa.ReduceOp.max)
            nc.sync.dma_start(out=ccw_in[:], in_=wmax_p[0:1, :])
            nc.gpsimd.collective_compute(
                "AllGather", mybir.AluOpType.bypass, replica_groups=rg,
                ins=[ccw_in[:].opt()], outs=[ccw_out[:].opt()])
            gat_w = stats.tile([ncores, 1], f32)
            nc.sync.dma_start(out=gat_w, in_=ccw_out[:])
            gmax_w = stats.tile([ncores, 1], f32)
            nc.gpsimd.partition_all_reduce(gmax_w, gat_w, channels=ncores,
                                           reduce_op=bass_isa.ReduceOp.max)
            aw = gmax_w[0:1, 0:1]

            # ---------- scales ----------
            scw = stats.tile([1, 4], f32)
            scx = stats.tile([1, 4], f32)
            sx_t = stats.tile([1, 1], f32)
            sw_t = stats.tile([1, 1], f32)
            ds_t = stats.tile([1, 1], f32)
            dsc = stats.tile([1, 4], f32)

            def recip(dst, src, t0, t1):
                nc.vector.reciprocal(dst, src)
                nc.vector.tensor_tensor(out=t0, in0=src, in1=dst,
                                        op=mybir.AluOpType.mult)
                nc.vector.tensor_scalar(out=t1, in0=t0, scalar1=-1.0, scalar2=2.0,
                                        op0=mybir.AluOpType.mult,
                                        op1=mybir.AluOpType.add)
                nc.vector.tensor_tensor(out=dst, in0=dst, in1=t1,
                                        op=mybir.AluOpType.mult)

            recip(scx[0:1, 0:1], ax, scx[0:1, 1:2], scx[0:1, 2:3])
            nc.vector.tensor_scalar(out=sx_t, in0=scx[0:1, 0:1], scalar1=MAXV,
                                    scalar2=None, op0=mybir.AluOpType.mult)
            sxb = stats.tile([P, 1], f32)
            nc.gpsimd.partition_broadcast(sxb, sx_t)

            recip(scw[0:1, 0:1], aw, scw[0:1, 1:2], scw[0:1, 2:3])
            nc.vector.tensor_scalar(out=sw_t, in0=scw[0:1, 0:1], scalar1=MAXV,
                                    scalar2=None, op0=mybir.AluOpType.mult)
            swb = stats.tile([P, 1], f32)
            nc.gpsimd.partition_broadcast(swb, sw_t)

            nc.vector.tensor_tensor(out=dsc[0:1, 0:1], in0=sx_t, in1=sw_t,
                                    op=mybir.AluOpType.mult)
            recip(ds_t, dsc[0:1, 0:1], dsc[0:1, 1:2], dsc[0:1, 2:3])
            dsb = stats.tile([P, 1], f32)
            nc.gpsimd.partition_broadcast(dsb, ds_t)

            # bias replicated into all partitions (fp32)
            bias_t = persist.tile([P, N], f32)
            bias_bcast = bass.AP(tensor=b_ext, offset=0, ap=[[0, P], [1, N]])
            nc.sync.dma_start(out=bias_t, in_=bias_bcast)

            # identity for TensorE transposes
            ident = persist.tile([P, P], f32)
            make_identity(nc, ident[:])
            ident_b = persist.tile([P, P], bf16)
            make_identity(nc, ident_b[:])

            # ---------- Phase C: quantize x + on-chip transpose ----------
            # chunk [128m, KC k] -> DVE round pass1 (in place) -> ACT pass2
            # (bf16 int8 values) -> PE 128x128 bf16 transposes -> PSUM ->
            # ACT copy into resident qxT [k-part, kt, m]
            qxT = persist.tile([P, KT, M], bf16)
            XG = 8                      # k-tiles per psum group
            for i in range(MB):
                for h in range(KH):
                    xc = xpool.tile([P, KC], f32)
                    nc.sync.dma_start(out=xc, in_=x_ext[i * P:(i + 1) * P, h * KC:(h + 1) * KC])
                    nc.vector.tensor_scalar(out=xc, in0=xc, scalar1=sxb,
                                            scalar2=MAGIC, op0=mybir.AluOpType.mult,
                                            op1=mybir.AluOpType.add)
                    qc = qxpool.tile([P, KC], bf16)
                    nc.scalar.activation(out=qc, in_=xc,
                                         func=mybir.ActivationFunctionType.Copy,
                                         bias=-MAGIC, scale=1.0)
                    for g in range(KC // (XG * P)):
                        px = psx.tile([P, XG, P], bf16, space="PSUM")
                        for j in range(XG):
                            ktl = g * XG + j
                            nc.tensor.transpose(px[:, j, :], qc[:, ktl * P:(ktl + 1) * P],
                                                ident_b[:])
                        kt0 = h * (KC // P) + g * XG
                        nc.scalar.copy(
                            out=qxT[:, kt0:kt0 + XG, i * P:(i + 1) * P], in_=px[:])
            # ---------- Phase D: weight stream + matmul ----------
            for nb in range(NB):
                qwT = qwtpool.tile([P, KT, 512], bf16)
                for s in range(4):          # 128-row sub-blocks of this n-block
                    n0 = nb * 512 + s * P
                    for h in range(KH):
                        wc = wpool.tile([P, KC], f32)
                        nc.sync.dma_start(out=wc, in_=w_ext[n0:n0 + P, h * KC:(h + 1) * KC])
                        for g in range(KC // (G * P)):   # groups of G k-tiles
                            ps = pstr.tile([P, G, P], f32, space="PSUM")
                            for j in range(G):
                                ktl = g * G + j
                                nc.tensor.transpose(
                                    ps[:, j, :], wc[:, ktl * P:(ktl + 1) * P],
                                    ident[:])
                            twt = twpool.tile([P, G, P], f32)
                            nc.scalar.activation(out=twt, in_=ps,
                                                 func=mybir.ActivationFunctionType.Copy,
                                                 bias=MAGIC, scale=swb)
                            kt0 = h * (KC // P) + g * G
                            nc.vector.tensor_scalar(
                                out=qwT[:, kt0:kt0 + G, s * P:(s + 1) * P],
                                in0=twt, scalar1=-MAGIC, scalar2=None,
                                op0=mybir.AluOpType.add)
                for mb in range(MB):
                    acc = psmm.tile([P, 512], f32, space="PSUM")
                    for kt in range(KT):
                        nc.tensor.matmul(
                            acc, qxT[:, kt, mb * P:(mb + 1) * P], qwT[:, kt, :],
                            start=(kt == 0), stop=(kt == KT - 1))
                    ob = obpool.tile([P, 512], f32)
                    nc.vector.scalar_tensor_tensor(
                        out=ob, in0=acc, scalar=dsb,
                        in1=bias_t[:, nb * 512:(nb + 1) * 512],
                        op0=mybir.AluOpType.mult, op1=mybir.AluOpType.add)
                    nc.sync.dma_start(
                        out=out_ext[mb * P:(mb + 1) * P, nb * 512:(nb + 1) * 512],
                        in_=ob)
    nc.compile()
    return nc


def shard_inputs(x, weight, bias, M=1024, N=4096, ncores=NCORES):
    xf = np.ascontiguousarray(np.asarray(x, dtype=np.float32).reshape(-1, x.shape[-1]))
    w = np.ascontiguousarray(np.asarray(weight, dtype=np.float32))
    b = np.ascontiguousarray(np.asarray(bias, dtype=np.float32))
    nsl = N // ncores
    in_maps = []
    for c in range(ncores):
        in_maps.append({
            "x": np.ascontiguousarray(xf[c * M:(c + 1) * M]),
            "w": w,
            "wslice": np.ascontiguousarray(w[c * nsl:(c + 1) * nsl]),
            "bias": b,
        })
    return in_maps


def _run(x, weight, bias, trace=False):
    from concourse.bass_utils import run_bass_kernel_spmd

    nc = build_graph()
    in_maps = shard_inputs(x, weight, bias)
    res = run_bass_kernel_spmd(nc, in_maps, core_ids=list(range(NCORES)),
                               trace=trace)
    outs = [res.results[c]["out"] for c in range(NCORES)]
    full = np.concatenate(outs, axis=0).reshape(FULL_B, FULL_S, FULL_N)
    return full.astype(np.float32), res


def kernel(x, weight, bias):
    out, _ = _run(x, weight, bias, trace=False)
    return out


# revision 20
# speedup vs baseline: 1.4060x; 1.3107x over previous
"""AdaPT Linear (int8 systolic fake-quant matmul) on 8 TRN2 NeuronCores.

Reference semantics (single device):
    amax_x = max|x|, amax_w = max|w|         (global scalars)
    sx = 127/amax_x, sw = 127/amax_w
    qx = round(x*sx)  (int8), qw = round(w*sw)  (int8)
    out = (qx @ qw.T)_int32 / (sx*sw) + bias

Distribution: data-parallel over x rows (8 x 1024 rows per core).

Layout strategy: the host stages k-major (transposed) copies of x and w, so
the TensorEngine's contraction-on-partitions layout falls out of plain DMA
loads and the PE does nothing but matmuls.  On each core:
  - amax over its xT shard + a 512-row slice of wT; two tiny AllGathers
    exchange per-core amax partials; scales = 127/amax via DVE reciprocal
    (one Newton step).
  - quantization = fp32 magic-number round (v*s + 1.5*2^23, subtract back;
    fp32 RNE makes this bit-exact round-half-to-even, matching jnp.round):
    DVE pass1 (in place), ACT pass2 (writes int8-valued bf16).
  - x: quantized once into a resident qxT [128k, kt, m] (8.4 MB).
  - w: quantized per 512-column output block into double-buffered qwT tiles,
    streamed from the host-staged wT.
  - matmul: lhsT = qxT k-tile [128k x 128m], rhs = qwT k-tile [128k x 512n],
    32-step accumulation into fp32 PSUM.  int8 products (<2^14) and sums
    (<2^24) are exact in the bf16 PE datapath, reproducing the int8 MAC.
  - epilogue: out = psum * (1/(sx*sw)) + bias in one DVE op, DMA out.

One NEFF; Tile generates all semaphores.
"""

import numpy as np

P = 128
MAGIC = 12582912.0  # 1.5 * 2**23: fp32 RNE round-to-int trick
MAXV = 127.0
NCORES = 8

# full-problem shapes (hardcoded per the task)
FULL_B, FULL_S, FULL_K = 4, 2048, 4096
FULL_N = 4096


def build_graph(M=1024, N=4096, K=4096, ncores=NCORES):
    """Build the SPMD Bass graph for one core (identical on all cores)."""
    import concourse.bass as bass
    import concourse.mybir as mybir
    import concourse.tile as tile
    from concourse import bacc, bass_isa

    assert M % P == 0 and K % P == 0 and N % 512 == 0
    NSL = N // ncores       # wT amax slice: N-rows? no: k-rows per core
    KT = K // P             # k tiles
    MB = M // P             # m blocks
    NB = N // 512           # n blocks of 512
    KSL = K // ncores       # k-rows of wT owned per core for amax

    f32 = mybir.dt.float32
    bf16 = mybir.dt.bfloat16

    nc = bacc.Bacc(None, num_devices=ncores)

    # host-staged k-major layouts
    xt_ext = nc.declare_dram_parameter("xT", [K, M], f32, isOutput=False)
    wt_ext = nc.declare_dram_parameter("wT", [K, N], f32, isOutput=False)
    wslt_ext = nc.declare_dram_parameter("wslT", [KSL, N], f32, isOutput=False)
    b_ext = nc.declare_dram_parameter("bias", [N], f32, isOutput=False)
    out_ext = nc.declare_dram_parameter("out", [M, N], f32, isOutput=True)

    ccw_in = nc.dram_tensor("ccw_in", [1, 1], f32)
    ccw_out = nc.dram_tensor("ccw_out", [ncores, 1], f32)
    ccx_in = nc.dram_tensor("ccx_in", [1, 1], f32)
    ccx_out = nc.dram_tensor("ccx_out", [ncores, 1], f32)

    xt_v = xt_ext[:].rearrange("(a p) m -> a p m", p=P)      # [KT, P, M]
    wslt_v = wslt_ext[:].rearrange("(a p) n -> a p n", p=P)  # [KSL/P, P, N]
    wt_v = wt_ext[:].rearrange("(a p) n -> a p n", p=P)      # [KT, P, N]

    with tile.TileContext(nc) as tc:
        with (
            tc.tile_pool(name="amax4k", bufs=2) as apool,
            tc.tile_pool(name="xq", bufs=3) as xpool,
            tc.tile_pool(name="wq", bufs=3) as wpool,
            tc.tile_pool(name="qxc", bufs=2) as qxpool,
            tc.tile_pool(name="persist", bufs=1) as persist,
            tc.tile_pool(name="qwt", bufs=2) as qwtpool,
            tc.tile_pool(name="ob", bufs=3) as obpool,
            tc.tile_pool(name="stats", bufs=1) as stats,
            tc.tile_pool(name="psum_mm", bufs=4, space="PSUM") as psmm,
        ):
            rg = [list(range(ncores))]

            # ---------- Phase A: amax partials + exchanges (x first) ----------
            # xT shard viewed [KT, P, M]; load 4 k-tiles per chunk
            XAC = 4
            xmaxes = stats.tile([P, KT // XAC], f32)
            for i in range(KT // XAC):
                xc = apool.tile([P, XAC, M], f32, tag="amax")
                nc.sync.dma_start(out=xc, in_=xt_v[XAC * i:XAC * (i + 1)].rearrange("a p m -> p a m"))
                nc.vector.tensor_reduce(
                    out=xmaxes[:, i:i + 1], in_=xc,
                    axis=mybir.AxisListType.XY, op=mybir.AluOpType.max,
                    apply_absolute_value=True)
            xmax_v = stats.tile([P, 1], f32)
            nc.vector.tensor_reduce(out=xmax_v, in_=xmaxes, axis=mybir.AxisListType.X,
                                    op=mybir.AluOpType.max)
            xmax_p = stats.tile([P, 1], f32)
            nc.gpsimd.partition_all_reduce(xmax_p, xmax_v, channels=P,
                                           reduce_op=bass_isa.ReduceOp.max)
            nc.sync.dma_start(out=ccx_in[:], in_=xmax_p[0:1, :])
            nc.gpsimd.collective_compute(
                "AllGather", mybir.AluOpType.bypass, replica_groups=rg,
                ins=[ccx_in[:].opt()], outs=[ccx_out[:].opt()])
            gat_x = stats.tile([ncores, 1], f32)
            nc.sync.dma_start(out=gat_x, in_=ccx_out[:])
            gmax_x = stats.tile([ncores, 1], f32)
            nc.gpsimd.partition_all_reduce(gmax_x, gat_x, channels=ncores,
                                           reduce_op=bass_isa.ReduceOp.max)
            ax = gmax_x[0:1, 0:1]

            # wT amax slice [KSL, N] viewed [KSL/P, P, N]
            wmaxes = stats.tile([P, KSL // P], f32)
            for i in range(KSL // P):
                wc = apool.tile([P, N], f32, tag="amax")
                nc.sync.dma_start(out=wc, in_=wslt_v[i])
                nc.vector.tensor_reduce(
                    out=wmaxes[:, i:i + 1], in_=wc,
                    axis=mybir.AxisListType.X, op=mybir.AluOpType.max,
                    apply_absolute_value=True)
            wmax_v = stats.tile([P, 1], f32)
            nc.vector.tensor_reduce(out=wmax_v, in_=wmaxes, axis=mybir.AxisListType.X,
                                    op=mybir.AluOpType.max)
            wmax_p = stats.tile([P, 1], f32)
            nc.gpsimd.partition_all_reduce(wmax_p, wmax_v, channels=P,
                                           reduce_op=bass_isa.ReduceOp.max)
            nc.sync.dma_start(out=ccw_in[:], in_=wmax_p[0:1, :])
            nc.gpsimd.collective_compute(
                "AllGather", mybir.AluOpType.bypass, replica_groups=rg,
                ins=[ccw_in[:].opt()], outs=[ccw_out[:].opt()])
            gat_w = stats.tile([ncores, 1], f32)
            nc.sync.dma_start(out=gat_w, in_=ccw_out[:])
            gmax_w = stats.tile([ncores, 1], f32)
            nc.gpsimd.partition_all_reduce(gmax_w, gat_w, channels=ncores,
                                           reduce_op=bass_isa.ReduceOp.max)
            aw = gmax_w[0:1, 0:1]

            # ---------- scales ----------
            scw = stats.tile([1, 4], f32)
            scx = stats.tile([1, 4], f32)
            sx_t = stats.tile([1, 1], f32)
            sw_t = stats.tile([1, 1], f32)
            ds_t = stats.tile([1, 1], f32)
            dsc = stats.tile([1, 4], f32)

            def recip(dst, src, t0, t1):
                nc.vector.reciprocal(dst, src)
                nc.vector.tensor_tensor(out=t0, in0=src, in1=dst,
                                        op=mybir.AluOpType.mult)
                nc.vector.tensor_scalar(out=t1, in0=t0, scalar1=-1.0, scalar2=2.0,
                                        op0=mybir.AluOpType.mult,
                                        op1=mybir.AluOpType.add)
                nc.vector.tensor_tensor(out=dst, in0=dst, in1=t1,
                                        op=mybir.AluOpType.mult)

            recip(scx[0:1, 0:1], ax, scx[0:1, 1:2], scx[0:1, 2:3])
            nc.vector.tensor_scalar(out=sx_t, in0=scx[0:1, 0:1], scalar1=MAXV,
                                    scalar2=None, op0=mybir.AluOpType.mult)
            sxb = stats.tile([P, 1], f32)
            nc.gpsimd.partition_broadcast(sxb, sx_t)

            recip(scw[0:1, 0:1], aw, scw[0:1, 1:2], scw[0:1, 2:3])
            nc.vector.tensor_scalar(out=sw_t, in0=scw[0:1, 0:1], scalar1=MAXV,
                                    scalar2=None, op0=mybir.AluOpType.mult)
            swb = stats.tile([P, 1], f32)
            nc.gpsimd.partition_broadcast(swb, sw_t)

            nc.vector.tensor_tensor(out=dsc[0:1, 0:1], in0=sx_t, in1=sw_t,
                                    op=mybir.AluOpType.mult)
            recip(ds_t, dsc[0:1, 0:1], dsc[0:1, 1:2], dsc[0:1, 2:3])
            dsb = stats.tile([P, 1], f32)
            nc.gpsimd.partition_broadcast(dsb, ds_t)

            # bias replicated into all partitions (fp32)
            bias_t = persist.tile([P, N], f32)
            bias_bcast = bass.AP(tensor=b_ext, offset=0, ap=[[0, P], [1, N]])
            nc.sync.dma_start(out=bias_t, in_=bias_bcast)

            # ---------- Phase C: quantize x straight into resident qxT ----------
            qxT = persist.tile([P, KT, M], bf16)
            for kt in range(KT):
                xcs = xpool.tile([P, M], f32)
                nc.sync.dma_start(out=xcs, in_=xt_v[kt])
                nc.vector.tensor_scalar(out=xcs, in0=xcs, scalar1=sxb,
                                        scalar2=MAGIC, op0=mybir.AluOpType.mult,
                                        op1=mybir.AluOpType.add)
                nc.scalar.activation(out=qxT[:, kt, :], in_=xcs,
                                     func=mybir.ActivationFunctionType.Copy,
                                     bias=-MAGIC, scale=1.0)

            # ---------- Phase D: per-block w quantize + matmul ----------
            for nb in range(NB):
                qwT = qwtpool.tile([P, KT, 512], bf16)
                for kt in range(KT):
                    wcs = wpool.tile([P, 512], f32)
                    nc.sync.dma_start(out=wcs,
                                      in_=wt_v[kt, :, nb * 512:(nb + 1) * 512])
                    nc.vector.tensor_scalar(out=wcs, in0=wcs, scalar1=swb,
                                            scalar2=MAGIC, op0=mybir.AluOpType.mult,
                                            op1=mybir.AluOpType.add)
                    nc.scalar.activation(out=qwT[:, kt, :], in_=wcs,
                                         func=mybir.ActivationFunctionType.Copy,
                                         bias=-MAGIC, scale=1.0)
                for mb in range(MB):
                    acc = psmm.tile([P, 512], f32, space="PSUM")
                    for kt in range(KT):
                        nc.tensor.matmul(
                            acc, qxT[:, kt, mb * P:(mb + 1) * P], qwT[:, kt, :],
                            start=(kt == 0), stop=(kt == KT - 1))
                    ob = obpool.tile([P, 512], f32)
                    nc.vector.scalar_tensor_tensor(
                        out=ob, in0=acc, scalar=dsb,
                        in1=bias_t[:, nb * 512:(nb + 1) * 512],
                        op0=mybir.AluOpType.mult, op1=mybir.AluOpType.add)
                    nc.sync.dma_start(
                        out=out_ext[mb * P:(mb + 1) * P, nb * 512:(nb + 1) * 512],
                        in_=ob)
    nc.compile()
    return nc


def shard_inputs(x, weight, bias, M=1024, K=4096, ncores=NCORES):
    xf = np.asarray(x, dtype=np.float32).reshape(-1, x.shape[-1])
    xT = np.ascontiguousarray(xf.T)                    # [K, 8M]
    wT = np.ascontiguousarray(np.asarray(weight, dtype=np.float32).T)  # [K, N]
    b = np.ascontiguousarray(np.asarray(bias, dtype=np.float32))
    ksl = K // ncores
    in_maps = []
    for c in range(ncores):
        in_maps.append({
            "xT": np.ascontiguousarray(xT[:, c * M:(c + 1) * M]),
            "wT": wT,
            "wslT": np.ascontiguousarray(wT[c * ksl:(c + 1) * ksl]),
            "bias": b,
        })
    return in_maps


def _run(x, weight, bias, trace=False):
    from concourse.bass_utils import run_bass_kernel_spmd

    nc = build_graph()
    in_maps = shard_inputs(x, weight, bias)
    res = run_bass_kernel_spmd(nc, in_maps, core_ids=list(range(NCORES)),
                               trace=trace)
    outs = [res.results[c]["out"] for c in range(NCORES)]
    full = np.concatenate(outs, axis=0).reshape(FULL_B, FULL_S, FULL_N)
    return full.astype(np.float32), res


def kernel(x, weight, bias):
    out, _ = _run(x, weight, bias, trace=False)
    return out


# revision 21
# speedup vs baseline: 1.4068x; 1.0006x over previous
"""AdaPT Linear (int8 systolic fake-quant matmul) on 8 TRN2 NeuronCores.

Reference semantics (single device):
    amax_x = max|x|, amax_w = max|w|         (global scalars)
    sx = 127/amax_x, sw = 127/amax_w
    qx = round(x*sx)  (int8), qw = round(w*sw)  (int8)
    out = (qx @ qw.T)_int32 / (sx*sw) + bias

Distribution: data-parallel over x rows (8 x 1024 rows per core).

Pipeline per core (one NEFF, Tile generates all semaphores):
  - amax: w-slice partials first (small, unblocks the weight path via a tiny
    AllGather early), then the x-shard partials + second AllGather; scales =
    127/amax via DVE reciprocal + one Newton step.
  - quantization = fp32 magic-number round (v*s + 1.5*2^23, subtract back;
    fp32 RNE makes this bit-exact round-half-to-even, matching jnp.round).
  - x: natural [m, k] 128-row strips so matmuls unlock per strip: DVE pass1
    (in place) -> ACT pass2 (int8-valued bf16) -> PE 128x128 bf16 transposes
    -> PSUM -> ACT copy into resident qxT [128k, kt, m].
  - w: the host stages wT (k-major), so w quantization is pure vector work:
    per 512-column output block, DVE pass1 + ACT pass2 straight into
    double-buffered qwT tiles.  No PE transposes for w.
  - matmul: lhsT = qxT k-tile [128k x 128m], rhs = qwT k-tile [128k x 512n],
    32-step accumulation into fp32 PSUM.  int8 products (<2^14) and sums
    (<2^24) are exact in the bf16 PE datapath, reproducing the int8 MAC.
  - epilogue: out = psum * (1/(sx*sw)) + bias in one DVE op, DMA out.
"""

import numpy as np

P = 128
MAGIC = 12582912.0  # 1.5 * 2**23: fp32 RNE round-to-int trick
MAXV = 127.0
NCORES = 8

# full-problem shapes (hardcoded per the task)
FULL_B, FULL_S, FULL_K = 4, 2048, 4096
FULL_N = 4096


def build_graph(M=1024, N=4096, K=4096, ncores=NCORES):
    """Build the SPMD Bass graph for one core (identical on all cores)."""
    import concourse.bass as bass
    import concourse.mybir as mybir
    import concourse.tile as tile
    from concourse import bacc, bass_isa
    from concourse.masks import make_identity

    assert M % P == 0 and K % P == 0 and N % 512 == 0
    KT = K // P             # k tiles
    MB = M // P             # m blocks (x strips)
    NB = N // 512           # n blocks of 512
    KSL = K // ncores       # k-rows of wT per core for amax
    XG = 8                  # k-tiles per x-transpose PSUM group

    f32 = mybir.dt.float32
    bf16 = mybir.dt.bfloat16

    nc = bacc.Bacc(None, num_devices=ncores)

    x_ext = nc.declare_dram_parameter("x", [M, K], f32, isOutput=False)
    wt_ext = nc.declare_dram_parameter("wT", [K, N], f32, isOutput=False)
    wslt_ext = nc.declare_dram_parameter("wslT", [KSL, N], f32, isOutput=False)
    b_ext = nc.declare_dram_parameter("bias", [N], f32, isOutput=False)
    out_ext = nc.declare_dram_parameter("out", [M, N], f32, isOutput=True)

    ccw_in = nc.dram_tensor("ccw_in", [1, 1], f32)
    ccw_out = nc.dram_tensor("ccw_out", [ncores, 1], f32)
    ccx_in = nc.dram_tensor("ccx_in", [1, 1], f32)
    ccx_out = nc.dram_tensor("ccx_out", [ncores, 1], f32)

    wslt_v = wslt_ext[:].rearrange("(a p) n -> a p n", p=P)  # [KSL/P, P, N]
    wt_v = wt_ext[:].rearrange("(a p) n -> a p n", p=P)      # [KT, P, N]

    with tile.TileContext(nc) as tc:
        with (
            tc.tile_pool(name="x4k", bufs=2) as xpool,       # [P, K] f32 chunks
            tc.tile_pool(name="wq", bufs=3) as wpool,        # [P, 512] f32 chunks
            tc.tile_pool(name="qxc", bufs=2) as qxpool,      # [P, K] bf16
            tc.tile_pool(name="persist", bufs=1) as persist,
            tc.tile_pool(name="qwt", bufs=2) as qwtpool,
            tc.tile_pool(name="ob", bufs=3) as obpool,
            tc.tile_pool(name="stats", bufs=1) as stats,
            tc.tile_pool(name="psum_mm", bufs=4, space="PSUM") as psmm,
            tc.tile_pool(name="psum_x", bufs=2, space="PSUM") as psx,
        ):
            rg = [list(range(ncores))]

            def amax_exchange(part_vec, cc_in, cc_out, gat, gmax):
                nc.sync.dma_start(out=cc_in[:], in_=part_vec[0:1, :])
                nc.gpsimd.collective_compute(
                    "AllGather", mybir.AluOpType.bypass, replica_groups=rg,
                    ins=[cc_in[:].opt()], outs=[cc_out[:].opt()])
                nc.sync.dma_start(out=gat, in_=cc_out[:])
                nc.gpsimd.partition_all_reduce(gmax, gat, channels=ncores,
                                               reduce_op=bass_isa.ReduceOp.max)

            # ---------- Phase A1: w-slice amax (small, first) ----------
            wmaxes = stats.tile([P, KSL // P], f32)
            for i in range(KSL // P):
                wc = xpool.tile([P, K], f32, tag="big")
                wcs = wc[:, 0:N]
                nc.sync.dma_start(out=wcs, in_=wslt_v[i])
                nc.vector.tensor_reduce(
                    out=wmaxes[:, i:i + 1], in_=wcs,
                    axis=mybir.AxisListType.X, op=mybir.AluOpType.max,
                    apply_absolute_value=True)
            wmax_v = stats.tile([P, 1], f32)
            nc.vector.tensor_reduce(out=wmax_v, in_=wmaxes, axis=mybir.AxisListType.X,
                                    op=mybir.AluOpType.max)
            wmax_p = stats.tile([P, 1], f32)
            nc.gpsimd.partition_all_reduce(wmax_p, wmax_v, channels=P,
                                           reduce_op=bass_isa.ReduceOp.max)
            gat_w = stats.tile([ncores, 1], f32)
            gmax_w = stats.tile([ncores, 1], f32)
            amax_exchange(wmax_p, ccw_in, ccw_out, gat_w, gmax_w)
            aw = gmax_w[0:1, 0:1]

            # ---------- Phase A2: x amax ----------
            xmaxes = stats.tile([P, MB], f32)
            for i in range(MB):
                xc = xpool.tile([P, K], f32, tag="big")
                nc.sync.dma_start(out=xc, in_=x_ext[i * P:(i + 1) * P, :])
                nc.vector.tensor_reduce(
                    out=xmaxes[:, i:i + 1], in_=xc,
                    axis=mybir.AxisListType.X, op=mybir.AluOpType.max,
                    apply_absolute_value=True)
            xmax_v = stats.tile([P, 1], f32)
            nc.vector.tensor_reduce(out=xmax_v, in_=xmaxes, axis=mybir.AxisListType.X,
                                    op=mybir.AluOpType.max)
            xmax_p = stats.tile([P, 1], f32)
            nc.gpsimd.partition_all_reduce(xmax_p, xmax_v, channels=P,
                                           reduce_op=bass_isa.ReduceOp.max)
            gat_x = stats.tile([ncores, 1], f32)
            gmax_x = stats.tile([ncores, 1], f32)
            amax_exchange(xmax_p, ccx_in, ccx_out, gat_x, gmax_x)
            ax = gmax_x[0:1, 0:1]

            # ---------- scales ----------
            scw = stats.tile([1, 4], f32)
            scx = stats.tile([1, 4], f32)
            sx_t = stats.tile([1, 1], f32)
            sw_t = stats.tile([1, 1], f32)
            ds_t = stats.tile([1, 1], f32)
            dsc = stats.tile([1, 4], f32)

            def recip(dst, src, t0, t1):
                nc.vector.reciprocal(dst, src)
                nc.vector.tensor_tensor(out=t0, in0=src, in1=dst,
                                        op=mybir.AluOpType.mult)
                nc.vector.tensor_scalar(out=t1, in0=t0, scalar1=-1.0, scalar2=2.0,
                                        op0=mybir.AluOpType.mult,
                                        op1=mybir.AluOpType.add)
                nc.vector.tensor_tensor(out=dst, in0=dst, in1=t1,
                                        op=mybir.AluOpType.mult)

            recip(scw[0:1, 0:1], aw, scw[0:1, 1:2], scw[0:1, 2:3])
            nc.vector.tensor_scalar(out=sw_t, in0=scw[0:1, 0:1], scalar1=MAXV,
                                    scalar2=None, op0=mybir.AluOpType.mult)
            swb = stats.tile([P, 1], f32)
            nc.gpsimd.partition_broadcast(swb, sw_t)

            recip(scx[0:1, 0:1], ax, scx[0:1, 1:2], scx[0:1, 2:3])
            nc.vector.tensor_scalar(out=sx_t, in0=scx[0:1, 0:1], scalar1=MAXV,
                                    scalar2=None, op0=mybir.AluOpType.mult)
            sxb = stats.tile([P, 1], f32)
            nc.gpsimd.partition_broadcast(sxb, sx_t)

            nc.vector.tensor_tensor(out=dsc[0:1, 0:1], in0=sx_t, in1=sw_t,
                                    op=mybir.AluOpType.mult)
            recip(ds_t, dsc[0:1, 0:1], dsc[0:1, 1:2], dsc[0:1, 2:3])
            dsb = stats.tile([P, 1], f32)
            nc.gpsimd.partition_broadcast(dsb, ds_t)

            # bias replicated into all partitions (fp32)
            bias_t = persist.tile([P, N], f32)
            bias_bcast = bass.AP(tensor=b_ext, offset=0, ap=[[0, P], [1, N]])
            nc.sync.dma_start(out=bias_t, in_=bias_bcast)

            ident_b = persist.tile([P, P], bf16)
            make_identity(nc, ident_b[:])

            # ---------- Phase C: x quantize + on-chip transpose, per strip ----------
            qxT = persist.tile([P, KT, M], bf16)
            for i in range(MB):
                xc = xpool.tile([P, K], f32, tag="big")
                nc.sync.dma_start(out=xc, in_=x_ext[i * P:(i + 1) * P, :])
                nc.vector.tensor_scalar(out=xc, in0=xc, scalar1=sxb,
                                        scalar2=MAGIC, op0=mybir.AluOpType.mult,
                                        op1=mybir.AluOpType.add)
                qc = qxpool.tile([P, K], bf16)
                nc.scalar.activation(out=qc, in_=xc,
                                     func=mybir.ActivationFunctionType.Copy,
                                     bias=-MAGIC, scale=1.0)
                for g in range(KT // XG):
                    px = psx.tile([P, XG, P], bf16, space="PSUM")
                    for j in range(XG):
                        ktl = g * XG + j
                        nc.tensor.transpose(px[:, j, :], qc[:, ktl * P:(ktl + 1) * P],
                                            ident_b[:])
                    nc.scalar.copy(
                        out=qxT[:, g * XG:(g + 1) * XG, i * P:(i + 1) * P],
                        in_=px[:])

            # ---------- Phase D: per-block w quantize (vector only) + matmul ----------
            for nb in range(NB):
                qwT = qwtpool.tile([P, KT, 512], bf16)
                for kt in range(KT):
                    wcs = wpool.tile([P, 512], f32)
                    nc.sync.dma_start(out=wcs,
                                      in_=wt_v[kt, :, nb * 512:(nb + 1) * 512])
                    nc.vector.tensor_scalar(out=wcs, in0=wcs, scalar1=swb,
                                            scalar2=MAGIC, op0=mybir.AluOpType.mult,
                                            op1=mybir.AluOpType.add)
                    nc.scalar.activation(out=qwT[:, kt, :], in_=wcs,
                                         func=mybir.ActivationFunctionType.Copy,
                                         bias=-MAGIC, scale=1.0)
                for mb in range(MB):
                    acc = psmm.tile([P, 512], f32, space="PSUM")
                    for kt in range(KT):
                        nc.tensor.matmul(
                            acc, qxT[:, kt, mb * P:(mb + 1) * P], qwT[:, kt, :],
                            start=(kt == 0), stop=(kt == KT - 1))
                    ob = obpool.tile([P, 512], f32)
                    nc.vector.scalar_tensor_tensor(
                        out=ob, in0=acc, scalar=dsb,
                        in1=bias_t[:, nb * 512:(nb + 1) * 512],
                        op0=mybir.AluOpType.mult, op1=mybir.AluOpType.add)
                    nc.sync.dma_start(
                        out=out_ext[mb * P:(mb + 1) * P, nb * 512:(nb + 1) * 512],
                        in_=ob)
    nc.compile()
    return nc


def shard_inputs(x, weight, bias, M=1024, K=4096, ncores=NCORES):
    xf = np.ascontiguousarray(np.asarray(x, dtype=np.float32).reshape(-1, x.shape[-1]))
    wT = np.ascontiguousarray(np.asarray(weight, dtype=np.float32).T)  # [K, N]
    b = np.ascontiguousarray(np.asarray(bias, dtype=np.float32))
    ksl = K // ncores
    in_maps = []
    for c in range(ncores):
        in_maps.append({
            "x": np.ascontiguousarray(xf[c * M:(c + 1) * M]),
            "wT": wT,
            "wslT": np.ascontiguousarray(wT[c * ksl:(c + 1) * ksl]),
            "bias": b,
        })
    return in_maps


def _run(x, weight, bias, trace=False):
    from concourse.bass_utils import run_bass_kernel_spmd

    nc = build_graph()
    in_maps = shard_inputs(x, weight, bias)
    res = run_bass_kernel_spmd(nc, in_maps, core_ids=list(range(NCORES)),
                               trace=trace)
    outs = [res.results[c]["out"] for c in range(NCORES)]
    full = np.concatenate(outs, axis=0).reshape(FULL_B, FULL_S, FULL_N)
    return full.astype(np.float32), res


def kernel(x, weight, bias):
    out, _ = _run(x, weight, bias, trace=False)
    return out


# revision 23
# speedup vs baseline: 1.4126x; 1.0041x over previous
"""AdaPT Linear (int8 systolic fake-quant matmul) on 8 TRN2 NeuronCores.

Reference semantics (single device):
    amax_x = max|x|, amax_w = max|w|         (global scalars)
    sx = 127/amax_x, sw = 127/amax_w
    qx = round(x*sx)  (int8), qw = round(w*sw)  (int8)
    out = (qx @ qw.T)_int32 / (sx*sw) + bias

Distribution: data-parallel over x rows (8 x 1024 rows per core).

Pipeline per core (one NEFF, Tile generates all semaphores):
  - amax: w-slice partials first (small, unblocks the weight path via a tiny
    AllGather early), then the x-shard partials + second AllGather; scales =
    127/amax via DVE reciprocal + one Newton step.
  - quantization = fp32 magic-number round (v*s + 1.5*2^23, subtract back;
    fp32 RNE makes this bit-exact round-half-to-even, matching jnp.round).
  - x: natural [m, k] 128-row strips so matmuls unlock per strip: DVE pass1
    (in place) -> ACT pass2 (int8-valued bf16) -> PE 128x128 bf16 transposes
    -> PSUM -> ACT copy into resident qxT [128k, kt, m].
  - w: the host stages wT (k-major), so w quantization is pure vector work:
    per 512-column output block, DVE pass1 + ACT pass2 straight into
    double-buffered qwT tiles.  No PE transposes for w.
  - matmul: lhsT = qxT k-tile [128k x 128m], rhs = qwT k-tile [128k x 512n],
    32-step accumulation into fp32 PSUM.  int8 products (<2^14) and sums
    (<2^24) are exact in the bf16 PE datapath, reproducing the int8 MAC.
  - epilogue: out = psum * (1/(sx*sw)) + bias in one DVE op, DMA out.
"""

import numpy as np

P = 128
MAGIC = 12582912.0  # 1.5 * 2**23: fp32 RNE round-to-int trick
MAXV = 127.0
NCORES = 8

# full-problem shapes (hardcoded per the task)
FULL_B, FULL_S, FULL_K = 4, 2048, 4096
FULL_N = 4096


def build_graph(M=1024, N=4096, K=4096, ncores=NCORES):
    """Build the SPMD Bass graph for one core (identical on all cores)."""
    import concourse.bass as bass
    import concourse.mybir as mybir
    import concourse.tile as tile
    from concourse import bacc, bass_isa
    from concourse.masks import make_identity

    assert M % P == 0 and K % P == 0 and N % 512 == 0
    KT = K // P             # k tiles
    MB = M // P             # m blocks (x strips)
    NB = N // 512           # n blocks of 512
    KSL = K // ncores       # k-rows of wT per core for amax
    XG = 8                  # k-tiles per x-transpose PSUM group

    f32 = mybir.dt.float32
    bf16 = mybir.dt.bfloat16

    nc = bacc.Bacc(None, num_devices=ncores)

    x_ext = nc.declare_dram_parameter("x", [M, K], f32, isOutput=False)
    wt_ext = nc.declare_dram_parameter("wT", [K, N], f32, isOutput=False)
    wslt_ext = nc.declare_dram_parameter("wslT", [KSL, N], f32, isOutput=False)
    b_ext = nc.declare_dram_parameter("bias", [N], f32, isOutput=False)
    out_ext = nc.declare_dram_parameter("out", [M, N], f32, isOutput=True)

    ccw_in = nc.dram_tensor("ccw_in", [1, 1], f32)
    ccw_out = nc.dram_tensor("ccw_out", [ncores, 1], f32)
    ccx_in = nc.dram_tensor("ccx_in", [1, 1], f32)
    ccx_out = nc.dram_tensor("ccx_out", [ncores, 1], f32)

    wslt_v = wslt_ext[:].rearrange("(a p) n -> a p n", p=P)  # [KSL/P, P, N]
    wt_v = wt_ext[:].rearrange("(a p) n -> a p n", p=P)      # [KT, P, N]

    with tile.TileContext(nc) as tc:
        KC = min(2048, K)
        KHH = K // KC
        WCC = min(KC, N)
        with (
            tc.tile_pool(name="x4k", bufs=2) as xpool,       # [P, KC] f32 chunks
            tc.tile_pool(name="wq", bufs=3) as wpool,        # [P, 512] f32 chunks
            tc.tile_pool(name="qxc", bufs=2) as qxpool,      # [P, K] bf16
            tc.tile_pool(name="persist", bufs=1) as persist,
            tc.tile_pool(name="qwt", bufs=3) as qwtpool,
            tc.tile_pool(name="ob", bufs=3) as obpool,
            tc.tile_pool(name="stats", bufs=1) as stats,
            tc.tile_pool(name="psum_mm", bufs=4, space="PSUM") as psmm,
            tc.tile_pool(name="psum_x", bufs=2, space="PSUM") as psx,
        ):
            rg = [list(range(ncores))]

            def amax_exchange(part_vec, cc_in, cc_out, gat, gmax):
                nc.sync.dma_start(out=cc_in[:], in_=part_vec[0:1, :])
                nc.gpsimd.collective_compute(
                    "AllGather", mybir.AluOpType.bypass, replica_groups=rg,
                    ins=[cc_in[:].opt()], outs=[cc_out[:].opt()])
                nc.sync.dma_start(out=gat, in_=cc_out[:])
                nc.gpsimd.partition_all_reduce(gmax, gat, channels=ncores,
                                               reduce_op=bass_isa.ReduceOp.max)

            # ---------- Phase A1: w-slice amax (small, first) ----------
            wmaxes = stats.tile([P, (KSL // P) * (N // WCC)], f32)
            for i in range(KSL // P):
                for h in range(N // WCC):
                    wcs = xpool.tile([P, KC], f32, tag="big")
                    nc.sync.dma_start(out=wcs[:, 0:WCC], in_=wslt_v[i, :, h * WCC:(h + 1) * WCC])
                    nc.vector.tensor_reduce(
                        out=wmaxes[:, i * (N // WCC) + h:i * (N // WCC) + h + 1],
                        in_=wcs[:, 0:WCC],
                        axis=mybir.AxisListType.X, op=mybir.AluOpType.max,
                        apply_absolute_value=True)
            wmax_v = stats.tile([P, 1], f32)
            nc.vector.tensor_reduce(out=wmax_v, in_=wmaxes, axis=mybir.AxisListType.X,
                                    op=mybir.AluOpType.max)
            wmax_p = stats.tile([P, 1], f32)
            nc.gpsimd.partition_all_reduce(wmax_p, wmax_v, channels=P,
                                           reduce_op=bass_isa.ReduceOp.max)
            gat_w = stats.tile([ncores, 1], f32)
            gmax_w = stats.tile([ncores, 1], f32)
            amax_exchange(wmax_p, ccw_in, ccw_out, gat_w, gmax_w)
            aw = gmax_w[0:1, 0:1]

            # ---------- Phase A2: x amax ----------
            xmaxes = stats.tile([P, MB * KHH], f32)
            for i in range(MB):
                for h in range(KHH):
                    xc = xpool.tile([P, KC], f32, tag="big")
                    nc.sync.dma_start(out=xc, in_=x_ext[i * P:(i + 1) * P, h * KC:(h + 1) * KC])
                    nc.vector.tensor_reduce(
                        out=xmaxes[:, i * KHH + h:i * KHH + h + 1], in_=xc,
                        axis=mybir.AxisListType.X, op=mybir.AluOpType.max,
                        apply_absolute_value=True)
            xmax_v = stats.tile([P, 1], f32)
            nc.vector.tensor_reduce(out=xmax_v, in_=xmaxes, axis=mybir.AxisListType.X,
                                    op=mybir.AluOpType.max)
            xmax_p = stats.tile([P, 1], f32)
            nc.gpsimd.partition_all_reduce(xmax_p, xmax_v, channels=P,
                                           reduce_op=bass_isa.ReduceOp.max)
            gat_x = stats.tile([ncores, 1], f32)
            gmax_x = stats.tile([ncores, 1], f32)
            amax_exchange(xmax_p, ccx_in, ccx_out, gat_x, gmax_x)
            ax = gmax_x[0:1, 0:1]

            # ---------- scales ----------
            scw = stats.tile([1, 4], f32)
            scx = stats.tile([1, 4], f32)
            sx_t = stats.tile([1, 1], f32)
            sw_t = stats.tile([1, 1], f32)
            ds_t = stats.tile([1, 1], f32)
            dsc = stats.tile([1, 4], f32)

            def recip(dst, src, t0, t1):
                nc.vector.reciprocal(dst, src)
                nc.vector.tensor_tensor(out=t0, in0=src, in1=dst,
                                        op=mybir.AluOpType.mult)
                nc.vector.tensor_scalar(out=t1, in0=t0, scalar1=-1.0, scalar2=2.0,
                                        op0=mybir.AluOpType.mult,
                                        op1=mybir.AluOpType.add)
                nc.vector.tensor_tensor(out=dst, in0=dst, in1=t1,
                                        op=mybir.AluOpType.mult)

            recip(scw[0:1, 0:1], aw, scw[0:1, 1:2], scw[0:1, 2:3])
            nc.vector.tensor_scalar(out=sw_t, in0=scw[0:1, 0:1], scalar1=MAXV,
                                    scalar2=None, op0=mybir.AluOpType.mult)
            swb = stats.tile([P, 1], f32)
            nc.gpsimd.partition_broadcast(swb, sw_t)

            recip(scx[0:1, 0:1], ax, scx[0:1, 1:2], scx[0:1, 2:3])
            nc.vector.tensor_scalar(out=sx_t, in0=scx[0:1, 0:1], scalar1=MAXV,
                                    scalar2=None, op0=mybir.AluOpType.mult)
            sxb = stats.tile([P, 1], f32)
            nc.gpsimd.partition_broadcast(sxb, sx_t)

            nc.vector.tensor_tensor(out=dsc[0:1, 0:1], in0=sx_t, in1=sw_t,
                                    op=mybir.AluOpType.mult)
            recip(ds_t, dsc[0:1, 0:1], dsc[0:1, 1:2], dsc[0:1, 2:3])
            dsb = stats.tile([P, 1], f32)
            nc.gpsimd.partition_broadcast(dsb, ds_t)

            # bias replicated into all partitions (fp32)
            bias_t = persist.tile([P, N], bf16)
            bias_bcast = bass.AP(tensor=b_ext, offset=0, ap=[[0, P], [1, N]])
            nc.gpsimd.dma_start(out=bias_t, in_=bias_bcast)

            ident_b = persist.tile([P, P], bf16)
            make_identity(nc, ident_b[:])

            # ---------- Phase C: x quantize + on-chip transpose, per strip ----------
            qxT = persist.tile([P, KT, M], bf16)
            KTH = KC // P      # k-tiles per half-chunk
            for i in range(MB):
                for h in range(KHH):
                    xc = xpool.tile([P, KC], f32, tag="big")
                    nc.sync.dma_start(out=xc, in_=x_ext[i * P:(i + 1) * P, h * KC:(h + 1) * KC])
                    nc.vector.tensor_scalar(out=xc, in0=xc, scalar1=sxb,
                                            scalar2=MAGIC, op0=mybir.AluOpType.mult,
                                            op1=mybir.AluOpType.add)
                    qc = qxpool.tile([P, KC], bf16)
                    nc.scalar.activation(out=qc, in_=xc,
                                         func=mybir.ActivationFunctionType.Copy,
                                         bias=-MAGIC, scale=1.0)
                    for g in range(KTH // XG):
                        px = psx.tile([P, XG, P], bf16, space="PSUM")
                        for j in range(XG):
                            ktl = g * XG + j
                            nc.tensor.transpose(px[:, j, :], qc[:, ktl * P:(ktl + 1) * P],
                                                ident_b[:])
                        kt0 = h * KTH + g * XG
                        nc.scalar.copy(
                            out=qxT[:, kt0:kt0 + XG, i * P:(i + 1) * P],
                            in_=px[:])

            # ---------- Phase D: per-block w quantize (vector only) + matmul ----------
            for nb in range(NB):
                qwT = qwtpool.tile([P, KT, 512], bf16)
                for kt in range(KT):
                    wcs = wpool.tile([P, 512], f32)
                    nc.sync.dma_start(out=wcs,
                                      in_=wt_v[kt, :, nb * 512:(nb + 1) * 512])
                    nc.vector.tensor_scalar(out=wcs, in0=wcs, scalar1=swb,
                                            scalar2=MAGIC, op0=mybir.AluOpType.mult,
                                            op1=mybir.AluOpType.add)
                    nc.scalar.activation(out=qwT[:, kt, :], in_=wcs,
                                         func=mybir.ActivationFunctionType.Copy,
                                         bias=-MAGIC, scale=1.0)
                for mb in range(MB):
                    acc = psmm.tile([P, 512], f32, space="PSUM")
                    for kt in range(KT):
                        nc.tensor.matmul(
                            acc, qxT[:, kt, mb * P:(mb + 1) * P], qwT[:, kt, :],
                            start=(kt == 0), stop=(kt == KT - 1))
                    ob = obpool.tile([P, 512], f32)
                    nc.vector.scalar_tensor_tensor(
                        out=ob, in0=acc, scalar=dsb,
                        in1=bias_t[:, nb * 512:(nb + 1) * 512],
                        op0=mybir.AluOpType.mult, op1=mybir.AluOpType.add)
                    nc.sync.dma_start(
                        out=out_ext[mb * P:(mb + 1) * P, nb * 512:(nb + 1) * 512],
                        in_=ob)
    nc.compile()
    return nc


def shard_inputs(x, weight, bias, M=1024, K=4096, ncores=NCORES):
    xf = np.ascontiguousarray(np.asarray(x, dtype=np.float32).reshape(-1, x.shape[-1]))
    wT = np.ascontiguousarray(np.asarray(weight, dtype=np.float32).T)  # [K, N]
    b = np.ascontiguousarray(np.asarray(bias, dtype=np.float32))
    ksl = K // ncores
    in_maps = []
    for c in range(ncores):
        in_maps.append({
            "x": np.ascontiguousarray(xf[c * M:(c + 1) * M]),
            "wT": wT,
            "wslT": np.ascontiguousarray(wT[c * ksl:(c + 1) * ksl]),
            "bias": b,
        })
    return in_maps


def _run(x, weight, bias, trace=False):
    from concourse.bass_utils import run_bass_kernel_spmd

    nc = build_graph()
    in_maps = shard_inputs(x, weight, bias)
    res = run_bass_kernel_spmd(nc, in_maps, core_ids=list(range(NCORES)),
                               trace=trace)
    outs = [res.results[c]["out"] for c in range(NCORES)]
    full = np.concatenate(outs, axis=0).reshape(FULL_B, FULL_S, FULL_N)
    return full.astype(np.float32), res


def kernel(x, weight, bias):
    out, _ = _run(x, weight, bias, trace=False)
    return out
